# revision 19
# baseline (speedup 1.0000x reference)
"""Trainium2 Bass kernel for nn_Candidate_Representation (gnn_message_passing).

Reference computation (P=128 passages, L=800, D=200, H=100, K=2 spans):
  M = P*K = 256 candidates
  S_Cs   [M,L,D] : S_p[p_m] masked to span [s_m, e_m] (zeros elsewhere)
  r_Cs   [M,H]   : tanh(S_p[p,s] @ Wb.T + S_p[p,e] @ We.T)
  enc    [M,L]   : passages[p, s:e+1] left-aligned, zero-padded
  V      [M,M-1] : pairwise Wv tanh(Wc r_i + Wo r_j), leave-one-out compacted
  tilda  [M,H]   : leave-one-out weighted sum of r via column-normalized exp

Sharding: data-parallel over the candidate axis — core c owns candidates
[32c, 32c+32) == passages [16c, 16c+16). The pairwise stage is sharded by
rows with two tiny AllGathers (r_Cs [256,100] and E [256,255]).
S_Cs is written with runtime-indexed scatter DMAs that only touch in-span
rows (outputs are delivered pre-zeroed by the SPMD runner).
"""

import time
import numpy as np

import concourse.bass as bass
import concourse.mybir as mybir
import concourse.tile as tile
from concourse import bass_utils
from concourse.masks import make_identity
from concourse.tile import ScopedClock

P, L, D, H, K = 128, 800, 200, 100, 2
M = P * K                    # 256 candidates
NC = 8                       # cores
MS = M // NC                 # 32 candidates per core
PS = P // NC                 # 16 passages per core
NCHUNK = 7                   # ceil(L/128) row chunks per passage
BIG = 1 << 24                # scatter index for masked rows (skipped via bounds)
F32 = mybir.dt.float32
I32 = mybir.dt.int32

# ---------------------------------------------------------------------------
# Patch: walrus here accepts only ONE sem wait per TPB_CTRL instruction, but
# TileContext's tail drain carries one wait per live semaphore. Split them.


def _patched_drain_and_barrier(self, tick_clock, wait_clock):
    nc = self.nc
    drain_inst = nc.sync.drain()
    wait_clock.add_sem_waits(
        drain_inst.ins, ScopedClock({None: tick_clock.global_clock})
    )
    mi = drain_inst.ins
    si = mi.sync_info
    waits = list(si.on_wait) if si is not None and si.on_wait else []
    if len(waits) > 1:
        si.on_wait = waits[:1]
        for w in waits[1:]:
            d2 = nc.sync.drain()
            if d2.ins.sync_info is None:
                d2.ins.sync_info = mybir.SyncInfo(on_wait=[], on_update=[])
            d2.ins.sync_info.on_wait = [w]

    nc.all_engine_barrier()
    assert self.sems is not None
    popped = nc._tile_sem_poison_stack.pop()
    assert popped is self._sem_poison
    nc.clear_and_free_semaphores(list(self.sems.allocated().values()))
    nc.all_engine_barrier()


tile.TileContext._drain_and_barrier = _patched_drain_and_barrier


def _split_multi_waits(nc):
    """Walrus here also caps DMA/compute instructions at one sem wait.
    Hoist extra waits onto single-wait NoOps inserted just before, on the
    same engine."""
    cnt = 0
    for f in nc.m.functions:
        for b in f.blocks:
            changed = False
            out = []
            for inst in b.instructions:
                si = inst.sync_info
                waits = list(si.on_wait) if si is not None and si.on_wait else []
                if len(waits) > 1:
                    changed = True
                    for w in waits[:-1]:
                        n = mybir.InstNoOp(name=f"I-wsplit-{cnt}", ins=[], outs=[])
                        cnt += 1
                        n.engine = inst.engine
                        n.sync_info = mybir.SyncInfo(on_wait=[w], on_update=[])
                        out.append(n)
                    si.on_wait = waits[-1:]
                out.append(inst)
            if changed:
                b.instructions = out


# ---------------------------------------------------------------------------


def build_program():
    nc = bass.Bass("TRN2", target_bir_lowering=False, debug=False, num_devices=NC)

    # ---- per-core inputs -------------------------------------------------
    sp = nc.dram_tensor("sp", [PS, L, D], F32, kind="ExternalInput")
    pp = nc.dram_tensor("pp", [PS, 2 * L], I32, kind="ExternalInput")  # padded
    wbt = nc.dram_tensor("wbt", [D, H], F32, kind="ExternalInput")   # Wb.T
    wet = nc.dram_tensor("wet", [D, H], F32, kind="ExternalInput")   # We.T
    wct = nc.dram_tensor("wct", [H, H], F32, kind="ExternalInput")   # Wc.T
    wot = nc.dram_tensor("wot", [H, H], F32, kind="ExternalInput")   # Wo.T
    wvt = nc.dram_tensor("wvt", [H, 1], F32, kind="ExternalInput")   # Wv[0][:,None]
    sbidx = nc.dram_tensor("sbidx", [MS, 1], I32, kind="ExternalInput")
    seidx = nc.dram_tensor("seidx", [MS, 1], I32, kind="ExternalInput")
    encidx = nc.dram_tensor("encidx", [MS, 1], I32, kind="ExternalInput")
    scol = nc.dram_tensor("scol", [MS, 1], I32, kind="ExternalInput")  # starts
    ecol = nc.dram_tensor("ecol", [MS, 1], I32, kind="ExternalInput")  # ends
    gcol = nc.dram_tensor("gcol", [MS, 1], F32, kind="ExternalInput")  # global ids
    srow = nc.dram_tensor("srow", [1, NCHUNK * MS], F32, kind="ExternalInput")
    erow = nc.dram_tensor("erow", [1, NCHUNK * MS], F32, kind="ExternalInput")

    # ---- per-core outputs ------------------------------------------------
    s_cs = nc.dram_tensor("s_cs", [MS, L, D], F32, kind="ExternalOutput")
    r_out = nc.dram_tensor("r_out", [MS, H], F32, kind="ExternalOutput")
    enc = nc.dram_tensor("enc", [MS, L], I32, kind="ExternalOutput")
    v_out = nc.dram_tensor("v_out", [MS, M - 1], F32, kind="ExternalOutput")
    til = nc.dram_tensor("til", [MS, H], F32, kind="ExternalOutput")

    # ---- collective bounce buffers --------------------------------------
    ag_r_in = nc.dram_tensor("ag_r_in", [MS, H], F32)
    ag_r_out = nc.dram_tensor("ag_r_out", [M, H], F32, addr_space="Shared")
    ag_e_in = nc.dram_tensor("ag_e_in", [MS, M - 1], F32)
    ag_e_out = nc.dram_tensor("ag_e_out", [M, M - 1], F32, addr_space="Shared")

    s_cs_flat = s_cs.ap().rearrange("m t d -> (m t) d")
    sp_rows = sp.ap().rearrange("p t d -> (p t) d")      # [12800, 200]
    pp_flat = pp.ap().rearrange("p t -> (p t)")[:, None]  # [25600, 1]

    with tile.TileContext(nc) as tc:
        with (
            tc.tile_pool(name="const", bufs=1) as cpool,
            tc.tile_pool(name="sbuf", bufs=2) as pool,
            tc.tile_pool(name="slab", bufs=3) as slabpool,
            tc.tile_pool(name="tpool", bufs=3) as tpool,
            tc.tile_pool(name="psum", bufs=2, space="PSUM") as psum,
            tc.tile_pool(name="psum1", bufs=1, space="PSUM") as psum1,
        ):
            ident = cpool.tile([128, 128], F32)
            make_identity(nc, ident[:])
            ones_col = cpool.tile([128, 1], F32)
            nc.vector.memset(ones_col[:], 1.0)
            ones_row = cpool.tile([1, 128], F32)
            nc.vector.memset(ones_row[:], 1.0)

            # ============ span mask build (for S_Cs) ======================
            # inspan[p, m, c] = 1.0 if s_m <= t <= e_m else 0.0, t = 128*c + p
            t_iota = cpool.tile([128, MS, NCHUNK], F32)
            nc.gpsimd.iota(t_iota[:], pattern=[[0, MS], [128, NCHUNK]],
                           base=0, channel_multiplier=1,
                           allow_small_or_imprecise_dtypes=True)
            # broadcast starts/ends rows across 128 partitions via DMA
            srep = cpool.tile([128, NCHUNK * MS], F32)
            erep = cpool.tile([128, NCHUNK * MS], F32)
            srow_b = bass.AP(srow, 0, [[0, 128], [1, NCHUNK * MS]])
            erow_b = bass.AP(erow, 0, [[0, 128], [1, NCHUNK * MS]])
            nc.sync.dma_start(out=srep[:], in_=srow_b)
            nc.sync.dma_start(out=erep[:], in_=erow_b)

            srep3 = srep[:].rearrange("p (m c) -> p m c", m=MS)
            erep3 = erep[:].rearrange("p (m c) -> p m c", m=MS)
            ge = pool.tile([128, MS, NCHUNK], F32, tag="maskbuild")
            nc.vector.tensor_tensor(out=ge[:], in0=t_iota[:], in1=srep3,
                                    op=mybir.AluOpType.is_ge)
            le = pool.tile([128, MS, NCHUNK], F32, tag="maskbuild2")
            nc.vector.tensor_tensor(out=le[:], in0=t_iota[:], in1=erep3,
                                    op=mybir.AluOpType.is_le)
            inspan = cpool.tile([128, MS, NCHUNK], F32)
            nc.vector.tensor_tensor(out=inspan[:], in0=ge[:], in1=le[:],
                                    op=mybir.AluOpType.mult)

            # ============ S_Cs streaming ==================================
            for pl in range(PS):
                slab = slabpool.tile([128, NCHUNK, D], F32, tag="slab")
                nc.sync.dma_start(
                    out=slab[:, 0:6, :],
                    in_=sp[pl, 0:768, :].rearrange("(c p) d -> p c d", p=128),
                )
                nc.sync.dma_start(out=slab[0:32, 6, :], in_=sp[pl, 768:800, :])
                for k in range(K):
                    m = K * pl + k
                    mcol = inspan[:, m, :]
                    mb = bass.AP(mcol.tensor, mcol.offset,
                                 list(mcol.ap) + [[0, D]])
                    masked = slabpool.tile([128, NCHUNK, D], F32, tag="masked")
                    nc.vector.tensor_tensor(out=masked[:], in0=slab[:],
                                            in1=mb, op=mybir.AluOpType.mult)
                    nc.sync.dma_start(
                        out=s_cs[m, 0:768, :].rearrange("(c p) d -> p c d",
                                                        p=128),
                        in_=masked[:, 0:6, :],
                    )
                    nc.sync.dma_start(out=s_cs[m, 768:800, :],
                                      in_=masked[0:32, 6, :])

            # ============ encoded_candidates ==============================
            encidx_t = pool.tile([MS, 1], I32, tag="encidx")
            nc.sync.dma_start(out=encidx_t[:], in_=encidx[:, :])
            enc_g = pool.tile([MS, L], I32, tag="encg")
            nc.gpsimd.indirect_dma_start(
                out=enc_g[:],
                out_offset=None,
                in_=pp_flat,
                in_offset=bass.IndirectOffsetOnAxis(ap=encidx_t[:, :1], axis=0),
            )
            scol_t = pool.tile([MS, 1], I32, tag="scol")
            ecol_t = pool.tile([MS, 1], I32, tag="ecol")
            nc.sync.dma_start(out=scol_t[:], in_=scol[:, :])
            nc.sync.dma_start(out=ecol_t[:], in_=ecol[:, :])
            len_col = pool.tile([MS, 1], I32, tag="lencol")
            nc.vector.tensor_tensor(out=len_col[:], in0=ecol_t[:], in1=scol_t[:],
                                    op=mybir.AluOpType.subtract)
            len_f = pool.tile([MS, 1], F32, tag="lenf")
            nc.vector.tensor_copy(out=len_f[:], in_=len_col[:])
            tl_iota = pool.tile([MS, L], F32, tag="tliota")
            nc.gpsimd.iota(tl_iota[:], pattern=[[1, L]], base=0, channel_multiplier=0,
                           allow_small_or_imprecise_dtypes=True)
            lmask = pool.tile([MS, L], I32, tag="lmask")
            nc.vector.tensor_scalar(out=lmask[:], in0=tl_iota[:],
                                    scalar1=len_f[:, :1], scalar2=None,
                                    op0=mybir.AluOpType.is_le)
            nc.vector.tensor_tensor(out=enc_g[:], in0=enc_g[:], in1=lmask[:],
                                    op=mybir.AluOpType.mult)
            nc.sync.dma_start(out=enc[:, :], in_=enc_g[:])

            # ============ r_Cs ============================================
            sbi_t = pool.tile([MS, 1], I32, tag="sbi")
            sei_t = pool.tile([MS, 1], I32, tag="sei")
            nc.sync.dma_start(out=sbi_t[:], in_=sbidx[:, :])
            nc.sync.dma_start(out=sei_t[:], in_=seidx[:, :])
            sb_t = pool.tile([MS, D], F32, tag="sb")
            se_t = pool.tile([MS, D], F32, tag="se")
            nc.gpsimd.indirect_dma_start(
                out=sb_t[:], out_offset=None, in_=sp_rows,
                in_offset=bass.IndirectOffsetOnAxis(ap=sbi_t[:, :1], axis=0))
            nc.gpsimd.indirect_dma_start(
                out=se_t[:], out_offset=None, in_=sp_rows,
                in_offset=bass.IndirectOffsetOnAxis(ap=sei_t[:, :1], axis=0))

            # transpose sb/se -> [D, MS] in two partition chunks (128 + 72)
            sbT_a = pool.tile([128, MS], F32, tag="sbTa")
            sbT_b = pool.tile([72, MS], F32, tag="sbTb")
            seT_a = pool.tile([128, MS], F32, tag="seTa")
            seT_b = pool.tile([72, MS], F32, tag="seTb")
            for src, dsts in ((sb_t, (sbT_a, sbT_b)), (se_t, (seT_a, seT_b))):
                pt_a = psum.tile([128, MS], F32, tag="tr")
                nc.tensor.transpose(out=pt_a[:], in_=src[:, 0:128], identity=ident[0:MS, 0:MS])
                nc.vector.tensor_copy(out=dsts[0][:], in_=pt_a[:])
                pt_b = psum.tile([72, MS], F32, tag="tr")
                nc.tensor.transpose(out=pt_b[:], in_=src[:, 128:200], identity=ident[0:MS, 0:MS])
                nc.vector.tensor_copy(out=dsts[1][:], in_=pt_b[:])

            wbt_a = cpool.tile([128, H], F32)
            wbt_b = cpool.tile([72, H], F32)
            wet_a = cpool.tile([128, H], F32)
            wet_b = cpool.tile([72, H], F32)
            nc.sync.dma_start(out=wbt_a[:], in_=wbt[0:128, :])
            nc.sync.dma_start(out=wbt_b[:], in_=wbt[128:200, :])
            nc.sync.dma_start(out=wet_a[:], in_=wet[0:128, :])
            nc.sync.dma_start(out=wet_b[:], in_=wet[128:200, :])

            r_psum = psum1.tile([MS, H], F32, tag="big")
            nc.tensor.matmul(out=r_psum[:], lhsT=sbT_a[:], rhs=wbt_a[:],
                             start=True, stop=False)
            nc.tensor.matmul(out=r_psum[:], lhsT=sbT_b[:], rhs=wbt_b[:],
                             start=False, stop=False)
            nc.tensor.matmul(out=r_psum[:], lhsT=seT_a[:], rhs=wet_a[:],
                             start=False, stop=False)
            nc.tensor.matmul(out=r_psum[:], lhsT=seT_b[:], rhs=wet_b[:],
                             start=False, stop=True)
            r_sb = pool.tile([MS, H], F32, tag="rsb")
            nc.scalar.activation(r_sb[:], r_psum[:], mybir.ActivationFunctionType.Tanh)
            nc.sync.dma_start(out=r_out[:, :], in_=r_sb[:])
            nc.sync.dma_start(out=ag_r_in[:, :], in_=r_sb[:])

            # ============ AllGather r ====================================
            nc.gpsimd.collective_compute(
                "AllGather", mybir.AluOpType.bypass,
                ins=[ag_r_in[:, :]], outs=[ag_r_out[:, :]],
                replica_groups=[list(range(NC))],
            )

            r_full0 = pool.tile([128, H], F32, tag="rf0")
            r_full1 = pool.tile([128, H], F32, tag="rf1")
            nc.sync.dma_start(out=r_full0[:], in_=ag_r_out[0:128, :])
            nc.sync.dma_start(out=r_full1[:], in_=ag_r_out[128:256, :])

            # rT_full [H, M]
            rT = pool.tile([H, M], F32, tag="rT")
            for i, rf in enumerate((r_full0, r_full1)):
                pt = psum.tile([H, 128], F32, tag="tr")
                nc.tensor.transpose(out=pt[:], in_=rf[:], identity=ident[:])
                nc.vector.tensor_copy(out=rT[:, i * 128:(i + 1) * 128], in_=pt[:])
            # rT_mine [H, MS] from local r
            rTm_p = psum.tile([H, MS], F32, tag="tr")
            nc.tensor.transpose(out=rTm_p[:], in_=r_sb[:], identity=ident[0:MS, 0:MS])
            rTm = pool.tile([H, MS], F32, tag="rTm")
            nc.vector.tensor_copy(out=rTm[:], in_=rTm_p[:])

            wct_t = cpool.tile([H, H], F32)
            wot_t = cpool.tile([H, H], F32)
            wvt_t = cpool.tile([H, 1], F32)
            nc.sync.dma_start(out=wct_t[:], in_=wct[:, :])
            nc.sync.dma_start(out=wot_t[:], in_=wot[:, :])
            nc.sync.dma_start(out=wvt_t[:], in_=wvt[:, :])

            oT_p = psum1.tile([H, M], F32, tag="big")
            nc.tensor.matmul(out=oT_p[:], lhsT=wot_t[:], rhs=rT[:],
                             start=True, stop=True)
            oT = pool.tile([H, M], F32, tag="oT")
            nc.vector.tensor_copy(out=oT[:], in_=oT_p[:])
            cT_p = psum.tile([H, MS], F32, tag="mm")
            nc.tensor.matmul(out=cT_p[:], lhsT=wct_t[:], rhs=rTm[:],
                             start=True, stop=True)
            cT = pool.tile([H, MS], F32, tag="cT")
            nc.vector.tensor_copy(out=cT[:], in_=cT_p[:])

            # ============ pairwise rows ===================================
            # V_full.T accumulated column-by-column (PE outputs must start
            # at partition 0), then transposed back to [i, j] layout.
            vfT0_p = psum1.tile([128, MS], F32, tag="vft0")
            vfT1_p = psum1.tile([128, MS], F32, tag="vft1")
            for il in range(MS):
                t_i = tpool.tile([H, M], F32, tag="ti")
                nc.scalar.activation(t_i[:], oT[:],
                                     mybir.ActivationFunctionType.Tanh,
                                     bias=cT[:, il:il + 1])
                nc.tensor.matmul(out=vfT0_p[:, il:il + 1], lhsT=t_i[:, 0:128],
                                 rhs=wvt_t[:], start=True, stop=True)
                nc.tensor.matmul(out=vfT1_p[:, il:il + 1], lhsT=t_i[:, 128:M],
                                 rhs=wvt_t[:], start=True, stop=True)
            vfull = pool.tile([MS, M], F32, tag="vfull")
            for i, vp in enumerate((vfT0_p, vfT1_p)):
                vs = pool.tile([128, MS], F32, tag="vfTsb")
                nc.vector.tensor_copy(out=vs[:], in_=vp[:])
                tp = psum.tile([MS, 128], F32, tag="tr")
                nc.tensor.transpose(out=tp[:], in_=vs[:], identity=ident[:])
                nc.vector.tensor_copy(out=vfull[:, i * 128:(i + 1) * 128],
                                      in_=tp[:])

            # leave-one-out compaction: V[i,b] = vfull[i, b + (b>=gid)]
            gcol_t = pool.tile([MS, 1], F32, tag="gcol")
            nc.sync.dma_start(out=gcol_t[:], in_=gcol[:, :])
            j_iota = pool.tile([MS, M - 1], F32, tag="jiota")
            nc.gpsimd.iota(j_iota[:], pattern=[[1, M - 1]], base=0,
                           channel_multiplier=0,
                           allow_small_or_imprecise_dtypes=True)
            ltm = pool.tile([MS, M - 1], mybir.dt.uint32, tag="ltm")
            nc.vector.tensor_scalar(out=ltm[:], in0=j_iota[:],
                                    scalar1=gcol_t[:, :1], scalar2=None,
                                    op0=mybir.AluOpType.is_lt)
            v_sb = pool.tile([MS, M - 1], F32, tag="vsb")
            nc.vector.select(v_sb[:], ltm[:], vfull[:, 0:M - 1], vfull[:, 1:M])
            nc.sync.dma_start(out=v_out[:, :], in_=v_sb[:])

            # E = exp(V)
            e_mine = pool.tile([MS, M - 1], F32, tag="emine")
            nc.scalar.activation(e_mine[:], v_sb[:],
                                 mybir.ActivationFunctionType.Exp)
            nc.sync.dma_start(out=ag_e_in[:, :], in_=e_mine[:])

            # ============ AllGather E ====================================
            nc.gpsimd.collective_compute(
                "AllGather", mybir.AluOpType.bypass,
                ins=[ag_e_in[:, :]], outs=[ag_e_out[:, :]],
                replica_groups=[list(range(NC))],
            )

            e_full0 = pool.tile([128, M - 1], F32, tag="ef0")
            e_full1 = pool.tile([128, M - 1], F32, tag="ef1")
            nc.sync.dma_start(out=e_full0[:], in_=ag_e_out[0:128, :])
            nc.sync.dma_start(out=e_full1[:], in_=ag_e_out[128:256, :])

            # column sums of E -> denomS [1, M-1]
            dsum_p = psum.tile([1, M - 1], F32, tag="mm")
            nc.tensor.matmul(out=dsum_p[:], lhsT=ones_col[:], rhs=e_full0[:],
                             start=True, stop=False)
            nc.tensor.matmul(out=dsum_p[:], lhsT=ones_col[:], rhs=e_full1[:],
                             start=False, stop=True)
            dsum = pool.tile([1, M - 1], F32, tag="dsumsb")
            nc.vector.tensor_copy(out=dsum[:], in_=dsum_p[:])
            # broadcast across 128 partitions
            drep_p = psum1.tile([128, M - 1], F32, tag="big")
            nc.tensor.matmul(out=drep_p[:], lhsT=ones_row[:],
                             rhs=dsum[:], start=True, stop=True)
            drep = pool.tile([128, M - 1], F32, tag="drep")
            nc.vector.tensor_copy(out=drep[:], in_=drep_p[:])

            # alpha over all rows (for alphaS)
            asum_p = psum.tile([1, M - 1], F32, tag="mm")
            for i, ef in enumerate((e_full0, e_full1)):
                d_sub = pool.tile([128, M - 1], F32, tag="dsub")
                nc.vector.tensor_tensor(out=d_sub[:], in0=drep[:], in1=ef[:],
                                        op=mybir.AluOpType.subtract)
                nc.vector.reciprocal(d_sub[:], d_sub[:])
                nc.vector.tensor_tensor(out=d_sub[:], in0=d_sub[:], in1=ef[:],
                                        op=mybir.AluOpType.mult)
                nc.tensor.matmul(out=asum_p[:], lhsT=ones_col[:], rhs=d_sub[:],
                                 start=(i == 0), stop=(i == 1))
            asum = pool.tile([1, M - 1], F32, tag="asumsb")
            nc.vector.tensor_copy(out=asum[:], in_=asum_p[:])

            # alpha for my rows from local E
            a_mine = pool.tile([MS, M - 1], F32, tag="amine")
            nc.vector.tensor_tensor(out=a_mine[:], in0=drep[0:MS, :], in1=e_mine[:],
                                    op=mybir.AluOpType.subtract)
            nc.vector.reciprocal(a_mine[:], a_mine[:])
            nc.vector.tensor_tensor(out=a_mine[:], in0=a_mine[:], in1=e_mine[:],
                                    op=mybir.AluOpType.mult)

            # s_c = alphaS_rep - a_mine
            asrep_p = psum.tile([MS, M - 1], F32, tag="mm")
            nc.tensor.matmul(out=asrep_p[:], lhsT=ones_row[:, 0:MS],
                             rhs=asum[:], start=True, stop=True)
            s_c = pool.tile([MS, M - 1], F32, tag="sc")
            nc.vector.tensor_tensor(out=s_c[:], in0=asrep_p[:], in1=a_mine[:],
                                    op=mybir.AluOpType.subtract)

            # scatter back to full layout with zero diagonal:
            # w[:, j] = s_c[:, j] (j < gid) | 0 (j == gid) | s_c[:, j-1] (j > gid)
            s_pad = pool.tile([MS, M + 1], F32, tag="spad")
            nc.vector.memset(s_pad[:], 0.0)
            nc.vector.tensor_copy(out=s_pad[:, 1:M], in_=s_c[:])
            jm_iota = pool.tile([MS, M], F32, tag="jmiota")
            nc.gpsimd.iota(jm_iota[:], pattern=[[1, M]], base=0,
                           channel_multiplier=0,
                           allow_small_or_imprecise_dtypes=True)
            ltm2 = pool.tile([MS, M], mybir.dt.uint32, tag="ltm2")
            nc.vector.tensor_scalar(out=ltm2[:], in0=jm_iota[:],
                                    scalar1=gcol_t[:, :1], scalar2=None,
                                    op0=mybir.AluOpType.is_lt)
            eqm = pool.tile([MS, M], F32, tag="eqm")
            nc.vector.tensor_scalar(out=eqm[:], in0=jm_iota[:],
                                    scalar1=gcol_t[:, :1], scalar2=None,
                                    op0=mybir.AluOpType.is_equal)
            w_full = pool.tile([MS, M], F32, tag="wfull")
            nc.vector.select(w_full[:], ltm2[:], s_pad[:, 1:M + 1], s_pad[:, 0:M])
            # zero diagonal: w *= (1 - eq)
            nc.vector.tensor_scalar(out=eqm[:], in0=eqm[:], scalar1=-1.0,
                                    scalar2=1.0, op0=mybir.AluOpType.mult,
                                    op1=mybir.AluOpType.add)
            nc.vector.tensor_tensor(out=w_full[:], in0=w_full[:], in1=eqm[:],
                                    op=mybir.AluOpType.mult)

            # tilda = w_full @ r_full : transpose w, two accumulating matmuls
            til_p = psum.tile([MS, H], F32, tag="mm")
            for i, rf in enumerate((r_full0, r_full1)):
                wT_p = psum.tile([128, MS], F32, tag="tr")
                nc.tensor.transpose(out=wT_p[:], in_=w_full[:, i * 128:(i + 1) * 128],
                                    identity=ident[0:MS, 0:MS])
                wT = pool.tile([128, MS], F32, tag="wT")
                nc.vector.tensor_copy(out=wT[:], in_=wT_p[:])
                nc.tensor.matmul(out=til_p[:], lhsT=wT[:], rhs=rf[:],
                                 start=(i == 0), stop=(i == 1))
            til_sb = pool.tile([MS, H], F32, tag="tilsb")
            nc.vector.tensor_copy(out=til_sb[:], in_=til_p[:])
            nc.sync.dma_start(out=til[:, :], in_=til_sb[:])

    _split_multi_waits(nc)
    return nc


_NC_CACHE = None


def _get_program():
    global _NC_CACHE
    if _NC_CACHE is None:
        _NC_CACHE = build_program()
    return _NC_CACHE


def _make_in_maps(S_p, spans, passages, Wb, We, Wc, Wo, Wv):
    S_p = np.ascontiguousarray(np.asarray(S_p, dtype=np.float32))
    spans_np = np.asarray(spans).astype(np.int64)
    pass_i32 = np.asarray(passages).astype(np.int32)
    Wb = np.asarray(Wb, np.float32); We = np.asarray(We, np.float32)
    Wc = np.asarray(Wc, np.float32); Wo = np.asarray(Wo, np.float32)
    Wv = np.asarray(Wv, np.float32)

    starts = spans_np[:, :, 0].reshape(M)   # [M]
    ends = spans_np[:, :, 1].reshape(M)
    p_idx = np.repeat(np.arange(M // K), K)

    wbt = np.ascontiguousarray(Wb.T)
    wet = np.ascontiguousarray(We.T)
    wct = np.ascontiguousarray(Wc.T)
    wot = np.ascontiguousarray(Wo.T)
    wvt = np.ascontiguousarray(Wv[0][:, None])

    pp_pad = np.zeros((P, 2 * L), np.int32)
    pp_pad[:, :L] = pass_i32

    in_maps = []
    for c in range(NC):
        mm = np.arange(MS * c, MS * (c + 1))
        pl = p_idx[mm] - PS * c            # local passage index [MS]
        s = starts[mm]; e = ends[mm]
        in_maps.append({
            "sp": S_p[PS * c:PS * (c + 1)],
            "pp": pp_pad[PS * c:PS * (c + 1)],
            "wbt": wbt, "wet": wet, "wct": wct, "wot": wot, "wvt": wvt,
            "sbidx": (pl * L + s).astype(np.int32)[:, None],
            "seidx": (pl * L + e).astype(np.int32)[:, None],
            "encidx": (pl * 2 * L + s).astype(np.int32)[:, None],
            "scol": s.astype(np.int32)[:, None],
            "ecol": e.astype(np.int32)[:, None],
            "gcol": mm.astype(np.float32)[:, None],
            "srow": np.repeat(s.astype(np.float32), NCHUNK)[None, :],
            "erow": np.repeat(e.astype(np.float32), NCHUNK)[None, :],
        })
    return in_maps


def kernel(S_p, spans, passages, Wb, We, Wc, Wo, Wv):
    enc_dtype = np.asarray(passages).dtype
    in_maps = _make_in_maps(S_p, spans, passages, Wb, We, Wc, Wo, Wv)
    nc = _get_program()
    res = bass_utils.run_bass_kernel_spmd(nc, in_maps, core_ids=list(range(NC)))

    S_Cs = np.concatenate([res.results[c]["s_cs"] for c in range(NC)], axis=0)
    r_Cs = np.concatenate([res.results[c]["r_out"] for c in range(NC)], axis=0)
    enc_out = np.concatenate([res.results[c]["enc"] for c in range(NC)], axis=0)
    V = np.concatenate([res.results[c]["v_out"] for c in range(NC)], axis=0)
    tilda = np.concatenate([res.results[c]["til"] for c in range(NC)], axis=0)
    return S_Cs, r_Cs, enc_out.astype(enc_dtype), V, tilda


def time_kernel(n_iters=30, **inputs):
    """Median per-iteration wall time (ns) of back-to-back pipelined
    executions of the prebuilt NEFF on all 8 cores (upper bound on HW time;
    host->device transfers are staged before the timed loop)."""
    import jax
    from jax.sharding import Mesh, NamedSharding, PartitionSpec
    from jax.experimental.shard_map import shard_map
    from concourse import bass2jax, mybir as mb

    nc = _get_program()
    in_maps = _make_in_maps(**inputs)
    bass2jax.install_neuronx_cc_hook()

    partition_name = nc.partition_id_tensor.name if nc.partition_id_tensor else None
    in_names, out_names, out_avals, zero_outs = [], [], [], []
    for alloc in nc.m.functions[0].allocations:
        if not isinstance(alloc, mb.MemoryLocationSet):
            continue
        name = alloc.memorylocations[0].name
        if alloc.kind == "ExternalInput":
            if name != partition_name:
                in_names.append(name)
        elif alloc.kind == "ExternalOutput":
            shape = tuple(alloc.tensor_shape)
            dtype = mb.dt.np(alloc.dtype)
            out_names.append(name)
            out_avals.append(jax.core.ShapedArray(shape, dtype))
            zero_outs.append(np.zeros(shape, dtype))
    n_params = len(in_names)
    all_in_names = in_names + out_names + ([partition_name] if partition_name else [])

    def _body(*args):
        operands = list(args)
        if partition_name is not None:
            operands.append(bass2jax.partition_id_tensor())
        return tuple(bass2jax._bass_exec_p.bind(
            *operands, out_avals=tuple(out_avals), in_names=tuple(all_in_names),
            out_names=tuple(out_names), lowering_input_output_aliases=(),
            sim_require_finite=True, sim_require_nnan=True, nc=nc))

    devices = jax.devices()[:NC]
    mesh = Mesh(np.asarray(devices), ("core",))
    nspec = (PartitionSpec("core"),) * (n_params + len(out_names))
    sharded = jax.jit(shard_map(_body, mesh=mesh, in_specs=nspec,
                                out_specs=(PartitionSpec("core"),) * len(out_names),
                                check_rep=False), keep_unused=True)
    sh = NamedSharding(mesh, PartitionSpec("core"))
    args = [
        jax.device_put(
            np.concatenate([np.asarray(in_maps[c][nm]) for c in range(NC)], axis=0), sh)
        for nm in in_names
    ] + [
        jax.device_put(np.zeros((NC * z.shape[0], *z.shape[1:]), z.dtype), sh)
        for z in zero_outs
    ]
    # warmup (compile)
    r = sharded(*args)
    jax.block_until_ready(r)
    times = []
    for _ in range(3):
        t0 = time.perf_counter()
        r = None
        for _ in range(n_iters):
            r = sharded(*args)
        jax.block_until_ready(r)
        times.append((time.perf_counter() - t0) / n_iters * 1e9)
    return min(times)
